# revision 6
# baseline (speedup 1.0000x reference)
"""GroupedQueryAttention Trainium2 kernel.

Reference computation (N=4, L=1024, E=2048, 32 heads of dim 64):
  energy[n,h,q,k] = sum_d Q[n,q,h*64+d] * K[n,k,h*64+d]
  attn = softmax(energy / sqrt(2048), axis=k)
  O[n,q,h*64+d]  = sum_k attn[n,h,q,k] * V[n,k,h*64+d]
  Y = O @ W_out.T + b_out
Sharding (8 cores): data-parallel over N (4) x tensor-parallel over head
halves (2); host sums the two fc_out partials per batch and adds the bias.

Per-core pipeline per head h (S^T orientation; softmax denominator via an
appended ones-column on V):
  S^T[k,q]   = KT_chunk.T @ QT      fp8e4 DoubleRow (Q,K quantized to e4m3,
                                    contraction 4x-duplicated onto 128
                                    partitions x2 pairs; /4 folded into the
                                    softmax scale).  2x PE throughput vs bf16.
  A'[k,q]    = exp(S^T * scale/4)   80% of tiles on ScalarE (act table),
                                    20% on VectorE as a factored minimax
                                    cubic (3 fused DVE instrs) - the exp is
                                    otherwise the single largest engine load.
  O'[e,q]    = sum_kc Vhat.T @ A'   bf16 (65 rows: 64 head dims + denom).
  OT[e,q]    = O'[0:64] * (1/den)   DVE, denom partition-broadcast via DMA.
  Y[l,o]     = sum_ec OT.T @ WT     bf16 fc_out partial.
"""

import sys

sys.path.insert(0, "/opt/trn_rl_repo")

import math

import numpy as np

import ml_dtypes

import concourse.bass as bass
import concourse.mybir as mybir
import concourse.tile as tile
from concourse import bass_utils
from concourse.bass_utils import run_bass_kernel_spmd


N, L, E = 4, 1024, 2048
HEADS, D = 32, 64
HPC = 16          # heads per core
EC = HPC * D      # e-columns per core (1024)
P = 128
SCALE = 1.0 / math.sqrt(float(E))
SCALE_EFF = SCALE / 4.0   # S matmul sums 4 duplicated copies of the d-dims
F32 = mybir.dt.float32
BF16 = mybir.dt.bfloat16
F8 = mybir.dt.float8e4
DR = mybir.MatmulPerfMode.DoubleRow
OP_ADD = mybir.AluOpType.add
OP_MULT = mybir.AluOpType.mult

# exp(y) ~= _K*(y + _A)*(y^2 + _B*y + _C) on y in [-0.95, 0.95] (max rel err
# 7.3e-3; softmax normalization cancels most of it - measured end-to-end
# impact < 4e-4).  Evaluated on DVE as 4 fused ops (only one PSUM read each):
#   y = x * SCALE_EFF          (PSUM f32 -> SBUF bf16)
#   q = (y + _B) * y
#   t = y * _K + _K*_A
#   a = (q + _C) * t
_K = 0.15615528109793714
_A = 1.7775033176157273
_B = 1.6081652151769075
_C = 3.596267161373759

# exp tiles sent to the DVE cubic instead of ScalarE: (h*8+kc) % 5 == 2
DVE_EXP = lambda idx: idx % 5 == 2


def _dedupe_ldweights(nc):
    """bf16/fp8 matmuls are emitted as explicit Ldweights+Matmult pairs, one
    pair per matmul.  Consecutive matmuls sharing the same stationary operand
    reload it needlessly; replace the redundant Ldweights by a NoOp that
    preserves its sync_info."""
    n_drop = 0
    for fn in nc.m.functions:
        stack = list(fn.blocks)
        while stack:
            bb = stack.pop()
            sub = getattr(bb, "blocks", None)
            if sub:
                stack.extend(sub)
            last_key = [None]
            new_insts = []
            for inst in bb.instructions:
                if str(inst.engine) not in ("EngineType.PE", "PE"):
                    new_insts.append(inst)
                    continue
                if inst.opcode == "Ldweights":
                    key = (
                        repr(inst.ins[0]),
                        str(inst.tile_position),
                        str(inst.tile_size),
                    )
                    if key == last_key[0]:
                        nop = mybir.InstNoOp(
                            name=inst.name,
                            engine=inst.engine,
                            ins=[],
                            outs=[],
                            sync_info=inst.sync_info,
                        )
                        new_insts.append(nop)
                        n_drop += 1
                    else:
                        last_key[0] = key
                        new_insts.append(inst)
                elif inst.opcode in ("Matmult", "NoOp", "EventSemaphore"):
                    new_insts.append(inst)
                else:
                    last_key[0] = None
                    new_insts.append(inst)
            bb.instructions = new_insts
    return n_drop


def _split_multi_waits(nc):
    """walrus in this image rejects >1 sem wait per instruction; hoist
    extra waits onto NoOps right before the instruction (same engine)."""
    n_split = 0
    for fn in nc.m.functions:
        stack = list(fn.blocks)
        while stack:
            bb = stack.pop()
            sub = getattr(bb, "blocks", None)
            if sub:
                stack.extend(sub)
            new_insts = []
            for inst in bb.instructions:
                si = inst.sync_info
                if si is not None and len(si.on_wait) > 1:
                    waits = list(si.on_wait)
                    for j, w in enumerate(waits[:-1]):
                        nop = mybir.InstNoOp(
                            name=f"{inst.name}_hw{j}",
                            engine=inst.engine,
                            ins=[],
                            outs=[],
                            sync_info=mybir.SyncInfo(on_wait=[w], on_update=[]),
                        )
                        new_insts.append(nop)
                        n_split += 1
                    si.on_wait = [waits[-1]]
                new_insts.append(inst)
            return_insts = new_insts
            bb.instructions = return_insts
    return n_split


def _build_program():
    nc = bass.Bass()
    qt = nc.declare_dram_parameter("qt", [HPC * P, 2, L], F8, isOutput=False)
    kt = nc.declare_dram_parameter("kt", [HPC * P, 2, L], F8, isOutput=False)
    vh = nc.declare_dram_parameter("vh", [L, HPC * 65], BF16, isOutput=False)
    wt = nc.declare_dram_parameter("wt", [EC, E], BF16, isOutput=False)
    yp = nc.declare_dram_parameter("yp", [L, E], F32, isOutput=True)

    with tile.TileContext(nc) as tc:
        with tc.tile_pool(name="persist", bufs=1) as persist:
            wt_sb = persist.tile([P, 8, E], BF16)
            ot = persist.tile([P, 8, L], BF16)
            rb_full = persist.tile([P, 8, L], BF16)
            den_d = persist.tile([HPC, L], F32, space="DRAM")
            rec_d = persist.tile([HPC, L], BF16, space="DRAM")
            with (
                tc.tile_pool(name="io", bufs=2) as io,
                tc.tile_pool(name="apool", bufs=3) as apool,
                tc.tile_pool(name="ps_s", bufs=2, space="PSUM") as ps_s,
                tc.tile_pool(name="ps_o", bufs=2, space="PSUM") as ps_o,
            ):
                for h in range(HPC):
                    hp, hi = h // 2, h % 2
                    po = hi * 64
                    qt2 = io.tile([P, 2, L], F8, tag="qt2")
                    kt2 = io.tile([P, 2, L], F8, tag="kt2")
                    vh2 = io.tile([P, 8, 65], BF16, tag="vh2")
                    nc.sync.dma_start(qt2[:], qt[h * P : (h + 1) * P, :, :])
                    nc.sync.dma_start(kt2[:], kt[h * P : (h + 1) * P, :, :])
                    nc.sync.dma_start(
                        vh2[:],
                        vh[:, h * 65 : (h + 1) * 65].rearrange(
                            "(c p) f -> p c f", p=P
                        ),
                    )
                    if h < 8:  # stage fc weights behind the head inputs
                        nc.sync.dma_start(
                            wt_sb[:, h, :], wt[h * P : (h + 1) * P, :]
                        )
                    o_ps = ps_o.tile([P, L], F32, tag="o")
                    for kc in range(8):
                        s_ps = ps_s.tile([P, L], F32, tag="s")
                        lhsT = kt2[:, :, kc * P : (kc + 1) * P]
                        for qc in range(2):
                            nc.tensor.matmul(
                                s_ps[:, qc * 512 : (qc + 1) * 512],
                                lhsT,
                                qt2[:, :, qc * 512 : (qc + 1) * 512],
                                start=True,
                                stop=True,
                                perf_mode=DR,
                            )
                        a_sb = apool.tile([P, L], BF16, tag="a")
                        if DVE_EXP(h * 8 + kc):
                            # factored cubic on VectorE (see constants above)
                            y_sb = apool.tile([P, L], BF16, tag="y")
                            q_sb = apool.tile([P, L], BF16, tag="q")
                            t_sb = apool.tile([P, L], BF16, tag="t")
                            with nc.allow_low_precision(
                                reason="bf16 softmax cubic; cancels in the "
                                "softmax normalization"
                            ):
                                nc.vector.tensor_scalar(
                                    y_sb[:], s_ps[:], float(SCALE_EFF), None,
                                    OP_MULT,
                                )
                                nc.vector.scalar_tensor_tensor(
                                    q_sb[:], y_sb[:], float(_B), y_sb[:],
                                    OP_ADD, OP_MULT,
                                )
                                nc.vector.tensor_scalar(
                                    t_sb[:], y_sb[:], float(_K), float(_K * _A),
                                    OP_MULT, OP_ADD,
                                )
                                nc.vector.scalar_tensor_tensor(
                                    a_sb[:], q_sb[:], float(_C), t_sb[:],
                                    OP_ADD, OP_MULT,
                                )
                        else:
                            nc.scalar.activation(
                                a_sb[:],
                                s_ps[:],
                                mybir.ActivationFunctionType.Exp,
                                scale=SCALE_EFF,
                            )
                        vsl = vh2[:, kc, :]
                        for qc in range(2):
                            nc.tensor.matmul(
                                o_ps[:65, qc * 512 : (qc + 1) * 512],
                                vsl,
                                a_sb[:, qc * 512 : (qc + 1) * 512],
                                start=(kc == 0),
                                stop=(kc == 7),
                            )
                    # evacuate PSUM fast: raw (unnormalized) head output
                    # and its softmax denominator row; normalize later.
                    nc.vector.tensor_copy(
                        out=ot[po : po + 64, hp, :], in_=o_ps[:64, :]
                    )
                    den_t = apool.tile([1, L], F32, tag="den")
                    nc.vector.tensor_copy(out=den_t[:], in_=o_ps[64:65, :])
                    nc.sync.dma_start(den_d[h : h + 1, :], den_t[:])
                    if hi == 1:
                        # normalize this finished pair's OT chunk in place,
                        # overlapped with the next heads' attention
                        j = hp
                        dsq = apool.tile([HPC, P], F32, tag="dsq")
                        nc.sync.dma_start(
                            dsq[:],
                            den_d[2 * j : 2 * j + 2, :].rearrange(
                                "h (a b) -> (h a) b", b=P
                            ),
                        )
                        rsq = apool.tile([HPC, P], BF16, tag="rsq")
                        with nc.allow_low_precision(
                            reason="softmax denom reciprocal to bf16; "
                            "0.4% relative is within the error budget"
                        ):
                            nc.vector.reciprocal(rsq[:], dsq[:])
                        nc.sync.dma_start(
                            rec_d[2 * j : 2 * j + 2, :].rearrange(
                                "h (a b) -> (h a) b", b=P
                            ),
                            rsq[:],
                        )
                        for ii in range(2):
                            nc.sync.dma_start(
                                rb_full[ii * 64 : (ii + 1) * 64, j, :],
                                rec_d[
                                    2 * j + ii : 2 * j + ii + 1, :
                                ].to_broadcast((64, L)),
                            )
                        nc.vector.tensor_mul(
                            ot[:, j, :], ot[:, j, :], rb_full[:, j, :]
                        )

            with (
                tc.tile_pool(name="ysb", bufs=2) as ysbp,
                tc.tile_pool(name="ps_y", bufs=2, space="PSUM") as ps_y,
            ):
                for lc in range(8):
                    y_ps = ps_y.tile([P, E], F32, tag="y")
                    for ec in range(8):
                        lhsT = ot[:, ec, lc * P : (lc + 1) * P]
                        for oc in range(4):
                            nc.tensor.matmul(
                                y_ps[:, oc * 512 : (oc + 1) * 512],
                                lhsT,
                                wt_sb[:, ec, oc * 512 : (oc + 1) * 512],
                                start=(ec == 0),
                                stop=(ec == 7),
                            )
                    y_sb = ysbp.tile([P, E], F32, tag="ysb")
                    nc.scalar.activation(
                        y_sb[:], y_ps[:], mybir.ActivationFunctionType.Copy
                    )
                    nc.sync.dma_start(yp[lc * P : (lc + 1) * P, :], y_sb[:])

    _dedupe_ldweights(nc)
    _split_multi_waits(nc)
    return nc


_NC_CACHE = []


def kernel(values, keys, queries, mask, W_out, b_out):
    values = np.asarray(values, dtype=np.float32)
    keys = np.asarray(keys, dtype=np.float32)
    queries = np.asarray(queries, dtype=np.float32)
    W_out = np.asarray(W_out, dtype=np.float32)
    b_out = np.asarray(b_out, dtype=np.float32)

    if not _NC_CACHE:
        _NC_CACHE.append(_build_program())
    nc = _NC_CACHE[0]

    in_maps = []
    for c in range(8):
        n, half = c // 2, c % 2
        cols = slice(half * EC, half * EC + EC)
        qs = queries[n][:, cols].astype(ml_dtypes.float8_e4m3)
        ks = keys[n][:, cols].astype(ml_dtypes.float8_e4m3)
        # [HPC*128, 2, L]: 4 duplicated copies of each head's 64 d-rows
        # (2 partition halves x 2 DoubleRow pairs); /4 folded into SCALE_EFF.
        qt = np.empty((HPC, 2, 64, 2, L), dtype=ml_dtypes.float8_e4m3)
        kt = np.empty((HPC, 2, 64, 2, L), dtype=ml_dtypes.float8_e4m3)
        for h in range(HPC):
            qh = qs[:, h * 64 : (h + 1) * 64].T  # [64, L]
            kh = ks[:, h * 64 : (h + 1) * 64].T
            qt[h] = qh[None, :, None, :]
            kt[h] = kh[None, :, None, :]
        qt = qt.reshape(HPC * P, 2, L)
        kt = kt.reshape(HPC * P, 2, L)
        v = values[n][:, cols]
        vhat = np.empty((L, HPC * 65), dtype=ml_dtypes.bfloat16)
        for h in range(HPC):
            vhat[:, h * 65 : h * 65 + 64] = v[:, h * 64 : (h + 1) * 64]
            vhat[:, h * 65 + 64] = 1.0
        wt = np.ascontiguousarray(W_out[:, cols].T).astype(ml_dtypes.bfloat16)
        in_maps.append({"qt": qt, "kt": kt, "vh": vhat, "wt": wt})

    res = run_bass_kernel_spmd(nc, in_maps, list(range(8)))

    out = np.empty((N, L, E), dtype=np.float32)
    for n in range(N):
        out[n] = res.results[2 * n]["yp"] + res.results[2 * n + 1]["yp"] + b_out
    return out


# revision 20
# speedup vs baseline: 1.3003x; 1.3003x over previous
"""GroupedQueryAttention Trainium2 kernel.

Reference computation (N=4, L=1024, E=2048, 32 heads of dim 64):
  energy[n,h,q,k] = sum_d Q[n,q,h*64+d] * K[n,k,h*64+d]
  attn = softmax(energy / sqrt(2048), axis=k)
  O[n,q,h*64+d]  = sum_k attn[n,h,q,k] * V[n,k,h*64+d]
  Y = O @ W_out.T + b_out
Sharding (8 cores): data-parallel over N (4) x tensor-parallel over head
halves (2); host sums the two fc_out partials per batch and adds the bias.

Per-core pipeline per head h (S^T orientation; softmax denominator via an
appended ones-column on V):
  S^T[k,q]   = KT_chunk.T @ QT      fp8e4 DoubleRow (Q,K quantized to e4m3,
                                    contraction 4x-duplicated onto 128
                                    partitions x2 pairs; /4 folded into the
                                    softmax scale).  2x PE throughput vs bf16.
  A'[k,q]    = exp(S^T * scale/4)   80% of tiles on ScalarE (act table),
                                    20% on VectorE as a factored minimax
                                    cubic (3 fused DVE instrs) - the exp is
                                    otherwise the single largest engine load.
  O'[e,q]    = sum_kc Vhat.T @ A'   bf16 (65 rows: 64 head dims + denom).
  OT[e,q]    = O'[0:64] * (1/den)   DVE, denom partition-broadcast via DMA.
  Y[l,o]     = sum_ec OT.T @ WT     bf16 fc_out partial.
"""

import sys

sys.path.insert(0, "/opt/trn_rl_repo")

import math

import numpy as np

import ml_dtypes

import concourse.bass as bass
import concourse.mybir as mybir
import concourse.tile as tile
from concourse import bass_utils
from concourse.bass_utils import run_bass_kernel_spmd


N, L, E = 4, 1024, 2048
HEADS, D = 32, 64
HPC = 16          # heads per core
EC = HPC * D      # e-columns per core (1024)
P = 128
SCALE = 1.0 / math.sqrt(float(E))
SCALE_EFF = SCALE / 4.0   # S matmul sums 4 duplicated copies of the d-dims
F32 = mybir.dt.float32
BF16 = mybir.dt.bfloat16
F8 = mybir.dt.float8e4
DR = mybir.MatmulPerfMode.DoubleRow
OP_ADD = mybir.AluOpType.add
OP_MULT = mybir.AluOpType.mult

# ~43% of exp tiles run on VectorE via the exponent-bit trick (Schraudolph):
#   a = bitcast_bf16(int16(S * BT_MUL + BT_BIAS))
# one fused tensor_scalar per tile.  Piecewise-linear 2^z, max rel err ~4%
# (sawtooth in the mantissa); the softmax denominator cancels most of it -
# simulated end-to-end contribution ~1.2%, total ~1.4% vs the 2e-2 gate.
LOG2E = 1.4426950408889634
BT_DELTA = 0.0573
BT_MUL = SCALE_EFF * LOG2E * 128.0
BT_BIAS = 16256.0 - 128.0 * BT_DELTA
I16 = mybir.dt.int16

# exp tiles sent to the DVE bit-trick instead of ScalarE
DVE_EXP = lambda idx: idx % 3 == 1


def _dedupe_ldweights(nc):
    """bf16/fp8 matmuls are emitted as explicit Ldweights+Matmult pairs, one
    pair per matmul.  Consecutive matmuls sharing the same stationary operand
    reload it needlessly; replace the redundant Ldweights by a NoOp that
    preserves its sync_info."""
    n_drop = 0
    for fn in nc.m.functions:
        stack = list(fn.blocks)
        while stack:
            bb = stack.pop()
            sub = getattr(bb, "blocks", None)
            if sub:
                stack.extend(sub)
            last_key = [None]
            new_insts = []
            for inst in bb.instructions:
                if str(inst.engine) not in ("EngineType.PE", "PE"):
                    new_insts.append(inst)
                    continue
                if inst.opcode == "Ldweights":
                    key = (
                        repr(inst.ins[0]),
                        str(inst.tile_position),
                        str(inst.tile_size),
                    )
                    if key == last_key[0]:
                        nop = mybir.InstNoOp(
                            name=inst.name,
                            engine=inst.engine,
                            ins=[],
                            outs=[],
                            sync_info=inst.sync_info,
                        )
                        new_insts.append(nop)
                        n_drop += 1
                    else:
                        last_key[0] = key
                        new_insts.append(inst)
                elif inst.opcode in ("Matmult", "NoOp", "EventSemaphore"):
                    new_insts.append(inst)
                else:
                    last_key[0] = None
                    new_insts.append(inst)
            bb.instructions = new_insts
    return n_drop


def _split_multi_waits(nc):
    """walrus in this image rejects >1 sem wait per instruction; hoist
    extra waits onto NoOps right before the instruction (same engine)."""
    n_split = 0
    for fn in nc.m.functions:
        stack = list(fn.blocks)
        while stack:
            bb = stack.pop()
            sub = getattr(bb, "blocks", None)
            if sub:
                stack.extend(sub)
            new_insts = []
            for inst in bb.instructions:
                si = inst.sync_info
                if si is not None and len(si.on_wait) > 1:
                    waits = list(si.on_wait)
                    for j, w in enumerate(waits[:-1]):
                        nop = mybir.InstNoOp(
                            name=f"{inst.name}_hw{j}",
                            engine=inst.engine,
                            ins=[],
                            outs=[],
                            sync_info=mybir.SyncInfo(on_wait=[w], on_update=[]),
                        )
                        new_insts.append(nop)
                        n_split += 1
                    si.on_wait = [waits[-1]]
                new_insts.append(inst)
            return_insts = new_insts
            bb.instructions = return_insts
    return n_split


def _normalize_pair(nc, ot, rb_full, j):
    """Scale pair j's raw head outputs in ot by the broadcast reciprocal
    denominators, in place."""
    nc.vector.tensor_mul(ot[:, j, :], ot[:, j, :], rb_full[:, j, :])


def _build_program():
    nc = bass.Bass()
    qt = nc.declare_dram_parameter("qt", [HPC * P, 2, L], F8, isOutput=False)
    kt = nc.declare_dram_parameter("kt", [HPC * P, 2, L], F8, isOutput=False)
    vh = nc.declare_dram_parameter("vh", [L, HPC * 65], BF16, isOutput=False)
    wt = nc.declare_dram_parameter("wt", [EC, E], BF16, isOutput=False)
    yp = nc.declare_dram_parameter("yp", [L, E], F32, isOutput=True)

    with tile.TileContext(nc) as tc:
        with tc.tile_pool(name="persist", bufs=1) as persist:
            wt_sb = persist.tile([P, 8, E], BF16)
            ot = persist.tile([P, 8, L], BF16)
            rb_full = persist.tile([P, 8, L], BF16)
            den_d = persist.tile([HPC, L], BF16, space="DRAM")
            rec_d = persist.tile([HPC, L], BF16, space="DRAM")
            with (
                tc.tile_pool(name="io", bufs=2) as io,
                tc.tile_pool(name="apool", bufs=3) as apool,
                tc.tile_pool(name="ps_s", bufs=2, space="PSUM") as ps_s,
                tc.tile_pool(name="ps_o", bufs=2, space="PSUM") as ps_o,
            ):
                for h in range(HPC):
                    hp, hi = h // 2, h % 2
                    po = hi * 64
                    qt2 = io.tile([P, 2, L], F8, tag="qt2")
                    kt2 = io.tile([P, 2, L], F8, tag="kt2")
                    vh2 = io.tile([P, 8, 65], BF16, tag="vh2")
                    nc.sync.dma_start(qt2[:], qt[h * P : (h + 1) * P, :, :])
                    nc.sync.dma_start(kt2[:], kt[h * P : (h + 1) * P, :, :])
                    nc.sync.dma_start(
                        vh2[:],
                        vh[:, h * 65 : (h + 1) * 65].rearrange(
                            "(c p) f -> p c f", p=P
                        ),
                    )
                    if h < 8:  # stage fc weights behind the head inputs
                        nc.sync.dma_start(
                            wt_sb[:, h, :], wt[h * P : (h + 1) * P, :]
                        )
                    o_ps = ps_o.tile([P, L], F32, tag="o")
                    for kc in range(8):
                        s_ps = ps_s.tile([P, L], F32, tag="s")
                        lhsT = kt2[:, :, kc * P : (kc + 1) * P]
                        for qc in range(2):
                            nc.tensor.matmul(
                                s_ps[:, qc * 512 : (qc + 1) * 512],
                                lhsT,
                                qt2[:, :, qc * 512 : (qc + 1) * 512],
                                start=True,
                                stop=True,
                                perf_mode=DR,
                            )
                        a_sb = apool.tile([P, L], BF16, tag="a")
                        if DVE_EXP(h * 8 + kc):
                            # exponent-bit-trick exp on VectorE (one fused op)
                            with nc.allow_low_precision(
                                reason="bit-trick softmax exp; cancels in "
                                "the softmax normalization"
                            ):
                                nc.vector.tensor_scalar(
                                    a_sb[:].bitcast(I16), s_ps[:],
                                    float(BT_MUL), float(BT_BIAS),
                                    OP_MULT, OP_ADD,
                                )
                        else:
                            nc.scalar.activation(
                                a_sb[:],
                                s_ps[:],
                                mybir.ActivationFunctionType.Exp,
                                scale=SCALE_EFF,
                            )
                        vsl = vh2[:, kc, :]
                        for qc in range(2):
                            nc.tensor.matmul(
                                o_ps[:65, qc * 512 : (qc + 1) * 512],
                                vsl,
                                a_sb[:, qc * 512 : (qc + 1) * 512],
                                start=(kc == 0),
                                stop=(kc == 7),
                            )
                    # evacuate PSUM fast: raw (unnormalized) head output
                    # and its softmax denominator row; normalize later.
                    nc.vector.tensor_copy(
                        out=ot[po : po + 64, hp, :], in_=o_ps[:64, :]
                    )
                    den_t = apool.tile([1, L], BF16, tag="den")
                    with nc.allow_low_precision(
                        reason="softmax denom in bf16; within error budget"
                    ):
                        nc.vector.tensor_copy(out=den_t[:], in_=o_ps[64:65, :])
                    nc.sync.dma_start(den_d[h : h + 1, :], den_t[:])
                    if hi == 1:
                        # kick off the reciprocal-broadcast chain for this
                        # pair; normalize the PREVIOUS pair (whose broadcast
                        # has certainly landed) to keep DVE off DMA waits.
                        j = hp
                        dsq = apool.tile([HPC, P], BF16, tag="dsq")
                        nc.sync.dma_start(
                            dsq[:],
                            den_d[2 * j : 2 * j + 2, :].rearrange(
                                "h (a b) -> (h a) b", b=P
                            ),
                        )
                        rsq = apool.tile([HPC, P], BF16, tag="rsq")
                        with nc.allow_low_precision(
                            reason="softmax denom reciprocal to bf16; "
                            "0.4% relative is within the error budget"
                        ):
                            nc.vector.reciprocal(rsq[:], dsq[:])
                        nc.sync.dma_start(
                            rec_d[2 * j : 2 * j + 2, :].rearrange(
                                "h (a b) -> (h a) b", b=P
                            ),
                            rsq[:],
                        )
                        for ii in range(2):
                            nc.sync.dma_start(
                                rb_full[ii * 64 : (ii + 1) * 64, j, :],
                                rec_d[
                                    2 * j + ii : 2 * j + ii + 1, :
                                ].to_broadcast((64, L)),
                            )
                        if j > 0:
                            _normalize_pair(nc, ot, rb_full, j - 1)
                _normalize_pair(nc, ot, rb_full, 7)

            with (
                tc.tile_pool(name="ysb", bufs=2) as ysbp,
                tc.tile_pool(name="ps_y", bufs=2, space="PSUM") as ps_y,
            ):
                for lc in range(8):
                    y_ps = ps_y.tile([P, E], F32, tag="y")
                    for ec in range(8):
                        lhsT = ot[:, ec, lc * P : (lc + 1) * P]
                        for oc in range(4):
                            nc.tensor.matmul(
                                y_ps[:, oc * 512 : (oc + 1) * 512],
                                lhsT,
                                wt_sb[:, ec, oc * 512 : (oc + 1) * 512],
                                start=(ec == 0),
                                stop=(ec == 7),
                            )
                    y_sb = ysbp.tile([P, E], F32, tag="ysb")
                    nc.scalar.activation(
                        y_sb[:], y_ps[:], mybir.ActivationFunctionType.Copy
                    )
                    nc.sync.dma_start(yp[lc * P : (lc + 1) * P, :], y_sb[:])

    _dedupe_ldweights(nc)
    _split_multi_waits(nc)
    return nc


_NC_CACHE = []


def kernel(values, keys, queries, mask, W_out, b_out):
    values = np.asarray(values, dtype=np.float32)
    keys = np.asarray(keys, dtype=np.float32)
    queries = np.asarray(queries, dtype=np.float32)
    W_out = np.asarray(W_out, dtype=np.float32)
    b_out = np.asarray(b_out, dtype=np.float32)

    if not _NC_CACHE:
        _NC_CACHE.append(_build_program())
    nc = _NC_CACHE[0]

    in_maps = []
    for c in range(8):
        n, half = c // 2, c % 2
        cols = slice(half * EC, half * EC + EC)
        qs = queries[n][:, cols].astype(ml_dtypes.float8_e4m3)
        ks = keys[n][:, cols].astype(ml_dtypes.float8_e4m3)
        # [HPC*128, 2, L]: 4 duplicated copies of each head's 64 d-rows
        # (2 partition halves x 2 DoubleRow pairs); /4 folded into SCALE_EFF.
        qt = np.empty((HPC, 2, 64, 2, L), dtype=ml_dtypes.float8_e4m3)
        kt = np.empty((HPC, 2, 64, 2, L), dtype=ml_dtypes.float8_e4m3)
        for h in range(HPC):
            qh = qs[:, h * 64 : (h + 1) * 64].T  # [64, L]
            kh = ks[:, h * 64 : (h + 1) * 64].T
            qt[h] = qh[None, :, None, :]
            kt[h] = kh[None, :, None, :]
        qt = qt.reshape(HPC * P, 2, L)
        kt = kt.reshape(HPC * P, 2, L)
        v = values[n][:, cols]
        vhat = np.empty((L, HPC * 65), dtype=ml_dtypes.bfloat16)
        for h in range(HPC):
            vhat[:, h * 65 : h * 65 + 64] = v[:, h * 64 : (h + 1) * 64]
            vhat[:, h * 65 + 64] = 1.0
        wt = np.ascontiguousarray(W_out[:, cols].T).astype(ml_dtypes.bfloat16)
        in_maps.append({"qt": qt, "kt": kt, "vh": vhat, "wt": wt})

    res = run_bass_kernel_spmd(nc, in_maps, list(range(8)))

    out = np.empty((N, L, E), dtype=np.float32)
    for n in range(N):
        out[n] = res.results[2 * n]["yp"] + res.results[2 * n + 1]["yp"] + b_out
    return out
